# revision 1
# baseline (speedup 1.0000x reference)
"""CalderaLinear fused kernel for 8 Trainium2 NeuronCores.

Math (reference): y = x @ Q^T + (x @ R^T) @ L^T + bias, with Q/L/R groupwise
int-dequantized (codes 0..15, group size 128).

Strategy:
  * Column-parallel over d_out: core c owns out-features [c*512, (c+1)*512).
  * On each core, first build W_c = Q_c^T + R^T @ L_c^T  ([d_in, 512]) on-chip:
    R and L^T are dequantized with DVE multiplies (codes and pre-broadcast
    scales arrive as exact bf16), R^T L^T accumulates on the PE into PSUM, and
    dequantized Q^T is added during PSUM eviction into the resident W tile.
  * Then y_c = x @ W_c + bias_c: x streams through as 128x512 pre-tiled
    blocks (host-side retiling gives one contiguous DMA per tile), W_c stays
    SBUF-resident, PSUM accumulates over the 32 k-tiles, bias is fused into
    the PSUM eviction.
  * All W-build operands are packed host-side into one contiguous
    [128, 51200] blob so the build phase streams in as a handful of large
    DMAs (the per-tensor version paid ~2.5us of cold-queue latency per small
    DMA at kernel start).
  * Host side only reshapes/transposes/casts and concatenates the 8 output
    shards: all dequant + matmul math runs on the NeuronCores.

Compute dtype is bf16 (codes 0..15 are exact; rel-err ~3e-3 vs fp32
reference, dominated by bf16 rounding of x and W). Set CALDERA_DTYPE=float32r
for the reduced-precision-fp32 PE mode (~2e-4 rel-err, ~15% slower).
"""

import os
import numpy as np
import ml_dtypes

P = 128
D_IN = 4096
D_OUT = 4096
TOK = 8192
RANK = 256
NCORES = 8
OC = D_OUT // NCORES      # 512 out features per core
KT = D_IN // P            # 32 contraction tiles
MS = 512                  # token slab
NS = TOK // MS            # 16 slabs
SUB = MS // P             # 4 psum sub-tiles per slab
KG = D_IN // 128          # 32 scale groups along d_in
RG = RANK // 128          # 2 scale groups along rank

# ---- W-build blob layout (columns, per partition), consumption-ordered ----
# [ ltv_j0 | lstb_j0 | ltv_j1 | lstb_j1 ]                    header: 2048
# then per R-chunk ch (8 chunks of 512 cols, covering W k-tiles 4ch..4ch+3):
#   [ rv_j0 | rs_j0 | rv_j1 | rs_j1 ]                        2048
#   [ qc_{4ch} | qb_{4ch} | qc_{4ch+1} | qb_{4ch+1} ]        2048
#   [ qc_{4ch+2} | qb_{4ch+2} | qc_{4ch+3} | qb_{4ch+3} ]    2048
RCH = 8
RCW = D_IN // RCH         # 512 R columns per chunk
HDR = RG * 2 * OC         # 2048
SEG = 3 * 2048            # per-chunk segment
WBCOLS = HDR + RCH * SEG  # 51200


def _rv_off(j, ch):
    return HDR + ch * SEG + j * 2 * RCW


def _rs_off(j, ch):
    return _rv_off(j, ch) + RCW


def _qc_off(k):
    return HDR + (k // 4) * SEG + 2048 + (k % 4) * 2 * OC


def _qb_off(k):
    return _qc_off(k) + OC

_module_cache = {}
last_result = None


def _build_module(dt_name):
    import concourse.mybir as mybir
    import concourse.tile as tile
    from concourse import bacc

    use_f32r = dt_name == "float32r"
    dt_c = getattr(mybir.dt, dt_name)
    f32 = mybir.dt.float32

    def mm(ap):
        return ap

    nc = bacc.Bacc(None, target_bir_lowering=False, debug=False)
    xt_d = nc.dram_tensor("xt", (NS, KT, P, MS), dt_c, kind="ExternalInput")
    wb_d = nc.dram_tensor("wb", (P, WBCOLS), dt_c, kind="ExternalInput")
    bias_d = nc.dram_tensor("biasv", (P, OC), f32, kind="ExternalInput")
    y_d = nc.dram_tensor("y", (TOK, OC), f32, kind="ExternalOutput")

    with tile.TileContext(nc) as tc:
        with (
            tc.tile_pool(name="const", bufs=1) as const,
            tc.tile_pool(name="wpool", bufs=1) as wpool,
            tc.tile_pool(name="xpool", bufs=16) as xpool,
            tc.tile_pool(name="qpool", bufs=4) as qpool,
            tc.tile_pool(name="ypool", bufs=8) as ypool,
            tc.tile_pool(name="ppool", bufs=6, space="PSUM") as ppool,
            tc.tile_pool(name="wbpool", bufs=2, space="PSUM") as wbpool,
        ):
            # In f32r mode only the header+R pieces stay SBUF-resident
            # (budget); Q pieces stream through qpool inside build_w instead.
            rseg = 2048 if use_f32r else SEG
            WB = const.tile([P, HDR + RCH * rseg], dt_c)
            bias_t = const.tile([P, OC], f32)

            def ltv(j):
                return WB[:, j * 2 * OC:j * 2 * OC + OC]

            def lst(j):
                return WB[:, j * 2 * OC + OC:(j + 1) * 2 * OC]

            def rv(j, ch):
                o = HDR + ch * rseg + j * 2 * RCW
                return WB[:, o:o + RCW]

            def rs(j, ch):
                o = HDR + ch * rseg + j * 2 * RCW + RCW
                return WB[:, o:o + RCW]

            def qc(k):
                return WB[:, _qc_off(k):_qc_off(k) + OC]

            def qb(k):
                return WB[:, _qb_off(k):_qb_off(k) + OC]

            # blob streams in consumption order as 0.5 MB pieces
            nc.sync.dma_start(WB[:, 0:HDR], wb_d[:, 0:HDR])
            for ch in range(RCH):
                for po in range(0, rseg, 2048):
                    nc.sync.dma_start(
                        WB[:, HDR + ch * rseg + po:HDR + ch * rseg + po + 2048],
                        wb_d[:, HDR + ch * SEG + po:HDR + ch * SEG + po + 2048],
                    )
            nc.sync.dma_start(bias_t[:], bias_d[:])

            # ---- dequantize L^T and R (codes x pre-broadcast scales).
            # R dequantizes in place over its code slice in the blob.
            LdT = const.tile([P, RG, OC], dt_c)
            for j in range(RG):
                nc.vector.tensor_mul(LdT[:, j, :], ltv(j), lst(j))

            def dequant_r(ch):
                # deferred per-chunk so the in-order DVE stream never blocks
                # the first W evictions on late R-chunk DMAs
                for j in range(RG):
                    nc.vector.tensor_mul(rv(j, ch), rv(j, ch), rs(j, ch))

            def rd(j, k):
                # dequantized R columns for W k-tile k (128 cols)
                base = HDR + (k // 4) * rseg + j * 2 * RCW + (k % 4) * P
                return WB[:, base:base + P]

            dequant_r(0)

            # ---- W_c = R^T @ L^T + Q^T, built one k-tile at a time.
            # The build is interleaved into slab 0's k-loop two tiles ahead
            # (build W[k+2] while slab 0 multiplies with W[k]) so the
            # DVE-bound build chain (~1.25us/k) hides under PE matmul work.
            Wt = wpool.tile([P, KT, OC], dt_c)

            def build_w(k):
                ps = wbpool.tile([P, OC], f32, tag="wb", name=f"wb{k}")
                for j in range(RG):
                    nc.tensor.matmul(
                        ps[:],
                        mm(rd(j, k)),
                        mm(LdT[:, j, :]),
                        start=(j == 0),
                        stop=(j == RG - 1),
                    )
                if use_f32r:
                    qt = qpool.tile([P, 2 * OC], dt_c, tag="qt")
                    nc.sync.dma_start(qt[:], wb_d[:, _qc_off(k):_qc_off(k) + 2 * OC])
                    qc_ap, qb_ap = qt[:, :OC], qt[:, OC:]
                else:
                    qc_ap, qb_ap = qc(k), qb(k)
                qdq = qpool.tile([P, OC], dt_c, tag="qd")
                nc.vector.tensor_mul(qdq[:], qc_ap, qb_ap)
                nc.vector.tensor_add(Wt[:, k, :], ps[:], qdq[:])

            def evict(psums, s):
                for sub in range(SUB):
                    yt = ypool.tile([P, OC], f32, tag="y", name=f"y{s}_{sub}")
                    nc.vector.tensor_add(yt[:], psums[sub][:], bias_t[:])
                    nc.scalar.dma_start(
                        y_d[s * MS + sub * P:s * MS + (sub + 1) * P, :], yt[:]
                    )

            for _k in range(3):
                build_w(_k)
            psums0 = [ppool.tile([P, OC], f32, tag="ps", name=f"ps0_{i}")
                      for i in range(SUB)]
            for k in range(KT):
                xt = xpool.tile([P, MS], dt_c, tag="x", name="xt0")
                nc.scalar.dma_start(xt[:], xt_d[0, k])
                for sub in range(SUB):
                    nc.tensor.matmul(
                        psums0[sub][:], mm(xt[:, sub * P:(sub + 1) * P]),
                        mm(Wt[:, k, :]), start=(k == 0), stop=(k == KT - 1),
                    )
                if k + 3 < KT:
                    if (k + 3) % (KT // RCH) == 0:
                        dequant_r((k + 3) // (KT // RCH))
                    build_w(k + 3)
                # KT//RCH == 4: chunk ch feeds W k-tiles 4ch..4ch+3
            evict(psums0, 0)

            for s in range(1, NS):
                psums = [
                    ppool.tile([P, OC], f32, tag="ps", name=f"ps{s}_{i}")
                    for i in range(SUB)
                ]
                for k in range(KT):
                    xt = xpool.tile([P, MS], dt_c, tag="x")
                    dma_eng = nc.sync if k % 2 == 0 else nc.scalar
                    dma_eng.dma_start(xt[:], xt_d[s, k])
                    for sub in range(SUB):
                        nc.tensor.matmul(
                            psums[sub][:],
                            mm(xt[:, sub * P:(sub + 1) * P]),
                            mm(Wt[:, k, :]),
                            start=(k == 0),
                            stop=(k == KT - 1),
                        )
                evict(psums, s)

    nc.compile()
    return nc


def kernel(x, q_values, q_scales, l_values, l_scales, r_values, r_scales, bias,
           _trace=False):
    from concourse.bass_utils import run_bass_kernel_spmd

    dt_name = os.environ.get("CALDERA_DTYPE", "bfloat16")
    np_in = ml_dtypes.bfloat16 if dt_name == "bfloat16" else np.float32

    if dt_name not in _module_cache:
        _module_cache[dt_name] = _build_module(dt_name)
    nc = _module_cache[dt_name]

    # host-side marshaling (layout + dtype only; all math runs on-device)
    x = np.asarray(x, dtype=np.float32)
    q_values = np.asarray(q_values)
    q_scales = np.asarray(q_scales)
    l_values = np.asarray(l_values)
    l_scales = np.asarray(l_scales)
    r_values = np.asarray(r_values)
    r_scales = np.asarray(r_scales)
    bias = np.asarray(bias)
    # xt[s, k, p, m] = x[s*MS + m, k*P + p]
    xt = np.ascontiguousarray(
        x.reshape(NS, MS, KT, P).transpose(0, 2, 3, 1)
    ).astype(np_in)
    rs_full = np.repeat(np.asarray(r_scales, np.float32), D_IN // KG, axis=1)
    rv_f = np.asarray(r_values, np.float32)

    in_maps = []
    for c in range(NCORES):
        sl = slice(c * OC, (c + 1) * OC)
        qt_c = q_values[sl].T.astype(np.float32)           # [D_IN, OC]
        qst_c = q_scales[sl].T.astype(np.float32)          # [KT, OC]
        ltv_c = l_values[sl].T.astype(np.float32)          # [RANK, OC]
        lst_c = l_scales[sl].T.astype(np.float32)          # [RG, OC]

        pieces = []
        for j in range(RG):
            pieces.append(ltv_c[j * P:(j + 1) * P, :])
            pieces.append(np.broadcast_to(lst_c[j].reshape(1, OC), (P, OC)))
        for ch in range(RCH):
            cs = slice(ch * RCW, (ch + 1) * RCW)
            for j in range(RG):
                pieces.append(rv_f[j * P:(j + 1) * P, cs])
                pieces.append(rs_full[j * P:(j + 1) * P, cs])
            for k in range(4 * ch, 4 * ch + 4):
                pieces.append(qt_c[k * P:(k + 1) * P, :])
                pieces.append(np.broadcast_to(qst_c[k].reshape(1, OC), (P, OC)))
        wb = np.concatenate(pieces, axis=1).astype(np_in)
        assert wb.shape == (P, WBCOLS)

        in_maps.append({
            "xt": xt,
            "wb": wb,
            "biasv": np.ascontiguousarray(
                np.broadcast_to(bias[sl].reshape(1, OC), (P, OC))
            ).astype(np.float32),
        })

    res = run_bass_kernel_spmd(
        nc, in_maps, core_ids=list(range(NCORES)), trace=_trace
    )
    global last_result
    last_result = res
    return np.concatenate([r["y"] for r in res.results], axis=1)



# revision 6
# speedup vs baseline: 1.1628x; 1.1628x over previous
"""CalderaLinear fused kernel for 8 Trainium2 NeuronCores (fp8 DoubleRow).

Math (reference): y = x @ Q^T + (x @ R^T) @ L^T + bias, with Q/L/R groupwise
int-dequantized (codes 0..15, group size 128 along the contraction dim).

Strategy (token-parallel / data-parallel):
  * Core c owns tokens [c*1024, (c+1)*1024) and computes its full y rows;
    outputs concatenate along axis 0. No replicated FLOPs, no collectives.
  * Error structure: the low-rank term dominates ||y|| by ~80x (xr entries
    have std ~325, amplified again through L), so base-path errors are
    diluted ~80x. The big base matmul (x @ Q^T, 275 of 310 GFLOP) therefore
    runs in fp8 e4m3 with MatmulPerfMode.DoubleRow (2 k-planes of 128
    contracted per pass = 2x bf16 PE throughput); int codes 0..15 are exact
    in e4m3, x and the code*scale products round at ~3% which lands ~4e-4
    on the output. The precision-critical low-rank path (xr = x @ R^T, then
    xr @ L^T) stays bf16, keeping total rel err ~3e-3 like the bf16 kernel.
  * All dequantization happens on-device (DVE): codes arrive as exact
    fp8/bf16 values, scales arrive pre-broadcast along partitions; a DVE
    multiply produces dequantized weights (in-place for Q/R/L).
  * Per core: prologue computes xr^T = R @ x^T on the PE (bf16, out-features
    of xr on PSUM partitions so no transpose is ever needed) and casts x to
    fp8; the main loop walks 4 out-feature quarters x 8 token groups, each
    PSUM group = 2 bf16 low-rank matmuls + 16x4 fp8 DoubleRow base matmuls,
    bias fused into the DVE eviction.
  * Q codes+scales (16.8 MB each, fp8) stream through a 2-quarter SBUF ring
    overlapped with compute; x streams per k-tile in the prologue.

PE budget/core: 65K cyc (xr^T) + 65K (low-rank) + 524K (base fp8) = 655K
cyc = 273 us at 2.4 GHz, vs 1.05M cyc (437 us) for the all-bf16 kernel.
"""

import numpy as np
import ml_dtypes

P = 128
D_IN = 4096
D_OUT = 4096
TOK = 8192
RANK = 256
NCORES = 8
TPC = TOK // NCORES       # 1024 tokens per core
KT = D_IN // P            # 32 k-tiles
K2 = KT // 2              # 16 double-k-tiles (DoubleRow contracts 256)
NQ = 4                    # out-feature quarters
QW = D_OUT // NQ          # 1024
OCC = 512                 # psum chunk width (one bank)
TG = TPC // P             # 8 token groups
RT = RANK // P            # 2 rank tiles

_module_cache = {}
last_result = None


def _build_module():
    import concourse.mybir as mybir
    import concourse.tile as tile
    from concourse import bacc

    f32 = mybir.dt.float32
    bf16 = mybir.dt.bfloat16
    fp8 = mybir.dt.float8e4
    DR = mybir.MatmulPerfMode.DoubleRow

    nc = bacc.Bacc(None, target_bir_lowering=False, debug=False)
    xb_d = nc.dram_tensor("xb", (KT, P, TPC), bf16, kind="ExternalInput")
    qv_d = nc.dram_tensor("qv", (NQ, K2, P, 2, QW), fp8, kind="ExternalInput")
    qs_d = nc.dram_tensor("qs", (NQ, K2, P, 2, QW), fp8, kind="ExternalInput")
    rv_d = nc.dram_tensor("rv", (P, KT, RANK), bf16, kind="ExternalInput")
    rs_d = nc.dram_tensor("rs", (P, KT, RANK), bf16, kind="ExternalInput")
    lv_d = nc.dram_tensor("lv", (P, RT, D_OUT), bf16, kind="ExternalInput")
    ls_d = nc.dram_tensor("ls", (P, RT, D_OUT), bf16, kind="ExternalInput")
    bias_d = nc.dram_tensor("biasv", (P, D_OUT), f32, kind="ExternalInput")
    y_d = nc.dram_tensor("y", (TPC, D_OUT), f32, kind="ExternalOutput")

    with tile.TileContext(nc) as tc:
        with (
            tc.tile_pool(name="const", bufs=1) as const,
            tc.tile_pool(name="xbp", bufs=4) as xbp,
            tc.tile_pool(name="qp", bufs=2) as qp,
            tc.tile_pool(name="qsp", bufs=3) as qsp,
            tc.tile_pool(name="yp", bufs=6) as yp,
            tc.tile_pool(name="xrps", bufs=4, space="PSUM") as xrps,
            tc.tile_pool(name="pp", bufs=4, space="PSUM") as pp,
        ):
            rv = const.tile([P, KT, RANK], bf16)    # becomes dequantized R^T
            lv = const.tile([P, RT, D_OUT], bf16)   # becomes dequantized L^T
            bias_t = const.tile([P, D_OUT], f32)
            xrT = const.tile([P, RT, TPC], bf16)    # xr^T: [rank, tokens]
            x8 = const.tile([P, K2, 2, TPC], fp8)
            # prologue scratch staged in the Q-quarter ring (slots are
            # reclaimed by quarters 0/1 once the scales are consumed)
            rs = qp.tile([P, KT, RANK], bf16, tag="q", name="rs")
            ls = qp.tile([P, RT, D_OUT], bf16, tag="q", name="ls")

            nc.sync.dma_start(rv[:], rv_d[:])
            nc.sync.dma_start(rs[:], rs_d[:])
            nc.scalar.dma_start(lv[:], lv_d[:])
            nc.scalar.dma_start(ls[:], ls_d[:])
            nc.scalar.dma_start(bias_t[:], bias_d[:])

            nc.vector.tensor_mul(rv[:], rv[:], rs[:])
            nc.vector.tensor_mul(lv[:], lv[:], ls[:])

            # ---- prologue: xr^T = R @ x^T (bf16) + cast x -> fp8
            xr_ps = [
                xrps.tile([P, OCC], f32, tag="xr", name=f"xr{i}")
                for i in range(4)
            ]
            for k in range(KT):
                xbt = xbp.tile([P, TPC], bf16, tag="xb")
                nc.gpsimd.dma_start(xbt[:], xb_d[k])
                for ts in range(2):
                    for rt in range(RT):
                        nc.tensor.matmul(
                            xr_ps[rt * 2 + ts][:],
                            rv[:, k, rt * P:(rt + 1) * P],
                            xbt[:, ts * OCC:(ts + 1) * OCC],
                            start=(k == 0), stop=(k == KT - 1),
                        )
                nc.vector.tensor_copy(x8[:, k // 2, k % 2, :], xbt[:])
            for rt in range(RT):
                for ts in range(2):
                    nc.vector.tensor_copy(
                        xrT[:, rt, ts * OCC:(ts + 1) * OCC],
                        xr_ps[rt * 2 + ts][:],
                    )

            # ---- main: per quarter, stream+dequant Q, then 8 token groups
            for q in range(NQ):
                qt = qp.tile([P, K2, 2, QW], fp8, tag="q", name=f"q{q}")
                for k2 in range(K2):
                    dma_q = nc.sync if k2 % 2 == 0 else nc.scalar
                    dma_s = nc.scalar if k2 % 2 == 0 else nc.sync
                    dma_q.dma_start(qt[:, k2], qv_d[q, k2])
                    qst = qsp.tile([P, 2, QW], fp8, tag="qs")
                    dma_s.dma_start(qst[:], qs_d[q, k2])
                    nc.vector.tensor_mul(qt[:, k2], qt[:, k2], qst[:])
                for t in range(TG):
                    psums = [
                        pp.tile([P, OCC], f32, tag="ps", name=f"ps{q}_{t}_{i}")
                        for i in range(2)
                    ]
                    for oc2 in range(2):
                        for rt in range(RT):
                            nc.tensor.matmul(
                                psums[oc2][:],
                                xrT[:, rt, t * P:(t + 1) * P],
                                lv[:, rt, q * QW + oc2 * OCC:
                                   q * QW + (oc2 + 1) * OCC],
                                start=(rt == 0), stop=False,
                            )
                    for k2 in range(K2):
                        last = k2 == K2 - 1
                        for oc2 in range(2):
                            for h in range(2):
                                off = oc2 * OCC + h * 256
                                nc.tensor.matmul(
                                    psums[oc2][:, h * 256:(h + 1) * 256],
                                    x8[:, k2, :, t * P:(t + 1) * P],
                                    qt[:, k2, :, off:off + 256],
                                    start=False, stop=last,
                                    perf_mode=DR,
                                )
                    for oc2 in range(2):
                        yt = yp.tile([P, OCC], f32, tag="y")
                        nc.vector.tensor_add(
                            yt[:], psums[oc2][:],
                            bias_t[:, q * QW + oc2 * OCC:
                                   q * QW + (oc2 + 1) * OCC],
                        )
                        nc.gpsimd.dma_start(
                            y_d[t * P:(t + 1) * P,
                                q * QW + oc2 * OCC:q * QW + (oc2 + 1) * OCC],
                            yt[:],
                        )

    nc.compile()
    return nc


def kernel(x, q_values, q_scales, l_values, l_scales, r_values, r_scales, bias,
           _trace=False):
    from concourse.bass_utils import run_bass_kernel_spmd

    if "mod" not in _module_cache:
        _module_cache["mod"] = _build_module()
    nc = _module_cache["mod"]

    bf = ml_dtypes.bfloat16
    f8 = ml_dtypes.float8_e4m3
    x = np.asarray(x, np.float32)
    qv = np.asarray(q_values)
    qsc = np.asarray(q_scales, np.float32)
    lvv = np.asarray(l_values)
    lsc = np.asarray(l_scales, np.float32)
    rvv = np.asarray(r_values)
    rsc = np.asarray(r_scales, np.float32)
    b = np.asarray(bias, np.float32)

    # host-side marshaling (layout + dtype only; all dequant/matmul math
    # runs on-device). d_in index i = k2*256 + pl*128 + p throughout.
    qv_h = np.ascontiguousarray(
        qv.T.reshape(K2, 2, P, NQ, QW).transpose(3, 0, 2, 1, 4)
    ).astype(np.float32).astype(f8)
    qs_small = qsc.T.reshape(K2, 2, NQ, QW).transpose(2, 0, 1, 3).astype(f8)
    qs_h = np.ascontiguousarray(
        np.broadcast_to(qs_small[:, :, None, :, :], (NQ, K2, P, 2, QW))
    )
    rv_h = np.ascontiguousarray(
        rvv.T.reshape(KT, P, RANK).transpose(1, 0, 2)
    ).astype(np.float32).astype(bf)
    rs_h = np.ascontiguousarray(
        np.broadcast_to(rsc.T.astype(bf)[None, :, :], (P, KT, RANK))
    )
    lv_h = np.ascontiguousarray(
        lvv.T.reshape(RT, P, D_OUT).transpose(1, 0, 2)
    ).astype(np.float32).astype(bf)
    ls_h = np.ascontiguousarray(
        np.broadcast_to(lsc.T.astype(bf)[None, :, :], (P, RT, D_OUT))
    )
    bias_h = np.ascontiguousarray(np.broadcast_to(b[None, :], (P, D_OUT)))

    in_maps = []
    for c in range(NCORES):
        xb_h = np.ascontiguousarray(
            x[c * TPC:(c + 1) * TPC].T.reshape(KT, P, TPC)
        ).astype(bf)
        in_maps.append({
            "xb": xb_h, "qv": qv_h, "qs": qs_h, "rv": rv_h, "rs": rs_h,
            "lv": lv_h, "ls": ls_h, "biasv": bias_h,
        })

    res = run_bass_kernel_spmd(
        nc, in_maps, core_ids=list(range(NCORES)), trace=_trace
    )
    global last_result
    last_result = res
    return np.concatenate([r["y"] for r in res.results], axis=0)


# revision 8
# speedup vs baseline: 1.2607x; 1.0842x over previous
"""CalderaLinear fused kernel for 8 Trainium2 NeuronCores (fp8 DoubleRow).

Math (reference): y = x @ Q^T + (x @ R^T) @ L^T + bias, with Q/L/R groupwise
int-dequantized (codes 0..15, group size 128 along the contraction dim).

Strategy (token-parallel / data-parallel):
  * Core c owns tokens [c*1024, (c+1)*1024) and computes its full y rows;
    outputs concatenate along axis 0. No replicated FLOPs, no collectives.
  * Error structure: the low-rank term dominates ||y|| by ~80x (xr entries
    have std ~325, amplified again through L), so base-path errors are
    diluted ~80x. The big base matmul (x @ Q^T, 275 of 310 GFLOP) therefore
    runs in fp8 e4m3 with MatmulPerfMode.DoubleRow (2 k-planes of 128
    contracted per pass = 2x bf16 PE throughput); int codes 0..15 are exact
    in e4m3, x and the code*scale products round at ~3% which lands ~4e-4
    on the output. The precision-critical low-rank path (xr = x @ R^T, then
    xr @ L^T) stays bf16, keeping total rel err ~3e-3 like the bf16 kernel.
  * All dequantization happens on-device (DVE): codes arrive as exact
    fp8/bf16 values, scales arrive pre-broadcast along partitions; a DVE
    multiply produces dequantized weights (in-place for Q/R/L).
  * Per core: prologue computes xr^T = R @ x^T on the PE (bf16, out-features
    of xr on PSUM partitions so no transpose is ever needed) and casts x to
    fp8; the main loop walks 4 out-feature quarters x 8 token groups, each
    PSUM group = 2 bf16 low-rank matmuls + 16x4 fp8 DoubleRow base matmuls,
    bias fused into the DVE eviction.
  * Q codes+scales (16.8 MB each, fp8) stream through a 2-quarter SBUF ring
    overlapped with compute; x streams per k-tile in the prologue.

PE budget/core: 65K cyc (xr^T) + 65K (low-rank) + 524K (base fp8) = 655K
cyc = 273 us at 2.4 GHz, vs 1.05M cyc (437 us) for the all-bf16 kernel.
"""

import numpy as np
import ml_dtypes

P = 128
D_IN = 4096
D_OUT = 4096
TOK = 8192
RANK = 256
NCORES = 8
TPC = TOK // NCORES       # 1024 tokens per core
KT = D_IN // P            # 32 k-tiles
K2 = KT // 2              # 16 double-k-tiles (DoubleRow contracts 256)
NQ = 4                    # out-feature quarters
QW = D_OUT // NQ          # 1024
OCC = 512                 # psum chunk width (one bank)
TG = TPC // P             # 8 token groups
RT = RANK // P            # 2 rank tiles

_module_cache = {}
last_result = None


def _build_module():
    import concourse.mybir as mybir
    import concourse.tile as tile
    from concourse import bacc

    f32 = mybir.dt.float32
    bf16 = mybir.dt.bfloat16
    fp8 = mybir.dt.float8e4
    DR = mybir.MatmulPerfMode.DoubleRow

    nc = bacc.Bacc(None, target_bir_lowering=False, debug=False)
    xb_d = nc.dram_tensor("xb", (KT, P, TPC), bf16, kind="ExternalInput")
    qv_d = nc.dram_tensor("qv", (NQ, K2, P, 2, QW), fp8, kind="ExternalInput")
    qs_d = nc.dram_tensor("qs", (NQ, K2, P, 2, QW), fp8, kind="ExternalInput")
    rv_d = nc.dram_tensor("rv", (P, KT, RANK), bf16, kind="ExternalInput")
    rs_d = nc.dram_tensor("rs", (P, KT, RANK), bf16, kind="ExternalInput")
    lv_d = nc.dram_tensor("lv", (P, RT, D_OUT), bf16, kind="ExternalInput")
    ls_d = nc.dram_tensor("ls", (P, RT, D_OUT), bf16, kind="ExternalInput")
    bias_d = nc.dram_tensor("biasv", (P, D_OUT), f32, kind="ExternalInput")
    y_d = nc.dram_tensor("y", (TPC, D_OUT), f32, kind="ExternalOutput")

    with tile.TileContext(nc) as tc:
        with (
            tc.tile_pool(name="const", bufs=1) as const,
            tc.tile_pool(name="xbp", bufs=4) as xbp,
            tc.tile_pool(name="qp", bufs=2) as qp,
            tc.tile_pool(name="qsp", bufs=3) as qsp,
            tc.tile_pool(name="yp", bufs=6) as yp,
            tc.tile_pool(name="xrps", bufs=4, space="PSUM") as xrps,
            tc.tile_pool(name="pp", bufs=4, space="PSUM") as pp,
        ):
            rv = const.tile([P, KT, RANK], bf16)    # becomes dequantized R^T
            lv = const.tile([P, RT, D_OUT], bf16)   # becomes dequantized L^T
            bias_t = const.tile([P, D_OUT], f32)
            xrT = const.tile([P, RT, TPC], bf16)    # xr^T: [rank, tokens]
            x8 = const.tile([P, K2, 2, TPC], fp8)
            # prologue scratch staged in the Q-quarter ring (slots are
            # reclaimed by quarters 0/1 once the scales are consumed)
            rs = qp.tile([P, KT, RANK], bf16, tag="q", name="rs")
            ls = qp.tile([P, RT, D_OUT], bf16, tag="q", name="ls")

            nc.sync.dma_start(rv[:], rv_d[:])
            nc.sync.dma_start(rs[:], rs_d[:])
            nc.scalar.dma_start(lv[:], lv_d[:])
            nc.scalar.dma_start(ls[:], ls_d[:])
            nc.scalar.dma_start(bias_t[:], bias_d[:])

            nc.vector.tensor_mul(rv[:], rv[:], rs[:])
            nc.vector.tensor_mul(lv[:], lv[:], ls[:])

            # quarter tiles allocated up-front in ring order (rs, ls, q0..q3);
            # DMAs/dequants are emitted interleaved with compute below.
            qts = [
                qp.tile([P, K2, 2, QW], fp8, tag="q", name=f"q{q}")
                for q in range(NQ)
            ]

            def fetch_q(q, k2):
                dma_q = nc.sync if k2 % 2 == 0 else nc.scalar
                dma_s = nc.scalar if k2 % 2 == 0 else nc.sync
                dma_q.dma_start(qts[q][:, k2], qv_d[q, k2])
                qst = qsp.tile([P, 2, QW], fp8, tag="qs", name=f"qs{q}_{k2}")
                dma_s.dma_start(qst[:], qs_d[q, k2])
                nc.vector.tensor_mul(qts[q][:, k2], qts[q][:, k2], qst[:])

            # ---- prologue: xr^T = R @ x^T (bf16) + cast x -> fp8
            xr_ps = [
                xrps.tile([P, OCC], f32, tag="xr", name=f"xr{i}")
                for i in range(4)
            ]
            for k in range(KT):
                xbt = xbp.tile([P, TPC], bf16, tag="xb")
                dma_x = nc.sync if k % 2 == 0 else nc.scalar
                dma_x.dma_start(xbt[:], xb_d[k])
                for ts in range(2):
                    for rt in range(RT):
                        nc.tensor.matmul(
                            xr_ps[rt * 2 + ts][:],
                            rv[:, k, rt * P:(rt + 1) * P],
                            xbt[:, ts * OCC:(ts + 1) * OCC],
                            start=(k == 0), stop=(k == KT - 1),
                        )
                nc.vector.tensor_copy(x8[:, k // 2, k % 2, :], xbt[:])
            # quarter 0 streams+dequants right behind the x casts; xr^T
            # evictions slot in after the first few so the PE's first
            # low-rank matmuls (needed ~7us into the main loop) aren't
            # blocked behind the whole dequant stream on the in-order DVE.
            for k2 in range(6):
                fetch_q(0, k2)
            for rt in range(RT):
                for ts in range(2):
                    nc.vector.tensor_copy(
                        xrT[:, rt, ts * OCC:(ts + 1) * OCC],
                        xr_ps[rt * 2 + ts][:],
                    )
            for k2 in range(6, K2):
                fetch_q(0, k2)

            # ---- main: 4 quarters x 8 token groups; base matmuls first in
            # each psum group (start), low-rank last (stop), so the group can
            # begin before xr^T is evicted; quarter q+1 is fetched/dequanted
            # two k2-tiles per token group to keep the in-order DVE ahead.
            for q in range(NQ):
                for t in range(TG):
                    psums = [
                        pp.tile([P, OCC], f32, tag="ps", name=f"ps{q}_{t}_{i}")
                        for i in range(2)
                    ]
                    for k2 in range(K2):
                        for oc2 in range(2):
                            nc.tensor.matmul(
                                psums[oc2][:],
                                x8[:, k2, :, t * P:(t + 1) * P],
                                qts[q][:, k2, :, oc2 * OCC:(oc2 + 1) * OCC],
                                start=(k2 == 0), stop=False,
                                perf_mode=DR,
                            )
                    for oc2 in range(2):
                        for rt in range(RT):
                            nc.tensor.matmul(
                                psums[oc2][:],
                                xrT[:, rt, t * P:(t + 1) * P],
                                lv[:, rt, q * QW + oc2 * OCC:
                                   q * QW + (oc2 + 1) * OCC],
                                start=False, stop=(rt == RT - 1),
                            )
                    if q + 1 < NQ:
                        fetch_q(q + 1, 2 * t)
                        fetch_q(q + 1, 2 * t + 1)
                    for oc2 in range(2):
                        yt = yp.tile([P, OCC], f32, tag="y")
                        nc.vector.tensor_add(
                            yt[:], psums[oc2][:],
                            bias_t[:, q * QW + oc2 * OCC:
                                   q * QW + (oc2 + 1) * OCC],
                        )
                        dma_y = nc.sync if (t + oc2) % 2 == 0 else nc.scalar
                        dma_y.dma_start(
                            y_d[t * P:(t + 1) * P,
                                q * QW + oc2 * OCC:q * QW + (oc2 + 1) * OCC],
                            yt[:],
                        )

    nc.compile()
    return nc


def kernel(x, q_values, q_scales, l_values, l_scales, r_values, r_scales, bias,
           _trace=False):
    from concourse.bass_utils import run_bass_kernel_spmd

    if "mod" not in _module_cache:
        _module_cache["mod"] = _build_module()
    nc = _module_cache["mod"]

    bf = ml_dtypes.bfloat16
    f8 = ml_dtypes.float8_e4m3
    x = np.asarray(x, np.float32)
    qv = np.asarray(q_values)
    qsc = np.asarray(q_scales, np.float32)
    lvv = np.asarray(l_values)
    lsc = np.asarray(l_scales, np.float32)
    rvv = np.asarray(r_values)
    rsc = np.asarray(r_scales, np.float32)
    b = np.asarray(bias, np.float32)

    # host-side marshaling (layout + dtype only; all dequant/matmul math
    # runs on-device). d_in index i = k2*256 + pl*128 + p throughout.
    qv_h = np.ascontiguousarray(
        qv.T.reshape(K2, 2, P, NQ, QW).transpose(3, 0, 2, 1, 4)
    ).astype(np.float32).astype(f8)
    qs_small = qsc.T.reshape(K2, 2, NQ, QW).transpose(2, 0, 1, 3).astype(f8)
    qs_h = np.ascontiguousarray(
        np.broadcast_to(qs_small[:, :, None, :, :], (NQ, K2, P, 2, QW))
    )
    rv_h = np.ascontiguousarray(
        rvv.T.reshape(KT, P, RANK).transpose(1, 0, 2)
    ).astype(np.float32).astype(bf)
    rs_h = np.ascontiguousarray(
        np.broadcast_to(rsc.T.astype(bf)[None, :, :], (P, KT, RANK))
    )
    lv_h = np.ascontiguousarray(
        lvv.T.reshape(RT, P, D_OUT).transpose(1, 0, 2)
    ).astype(np.float32).astype(bf)
    ls_h = np.ascontiguousarray(
        np.broadcast_to(lsc.T.astype(bf)[None, :, :], (P, RT, D_OUT))
    )
    bias_h = np.ascontiguousarray(np.broadcast_to(b[None, :], (P, D_OUT)))

    in_maps = []
    for c in range(NCORES):
        xb_h = np.ascontiguousarray(
            x[c * TPC:(c + 1) * TPC].T.reshape(KT, P, TPC)
        ).astype(bf)
        in_maps.append({
            "xb": xb_h, "qv": qv_h, "qs": qs_h, "rv": rv_h, "rs": rs_h,
            "lv": lv_h, "ls": ls_h, "biasv": bias_h,
        })

    res = run_bass_kernel_spmd(
        nc, in_maps, core_ids=list(range(NCORES)), trace=_trace
    )
    global last_result
    last_result = res
    return np.concatenate([r["y"] for r in res.results], axis=0)


# revision 16
# speedup vs baseline: 1.3962x; 1.1075x over previous
"""CalderaLinear fused kernel for 8 Trainium2 NeuronCores (fp8 DoubleRow).

Math (reference): y = x @ Q^T + (x @ R^T) @ L^T + bias, with Q/L/R groupwise
int-dequantized (codes 0..15, group size 128 along the contraction dim).

Strategy (token-parallel / data-parallel):
  * Core c owns tokens [c*1024, (c+1)*1024) and computes its full y rows;
    outputs concatenate along axis 0. No replicated FLOPs, no collectives.
  * Error structure: the low-rank term dominates ||y|| by ~80x (xr entries
    have std ~325, amplified again through L), so base-path errors are
    diluted ~80x. The big base matmul (x @ Q^T, 275 of 310 GFLOP) therefore
    runs in fp8 e4m3 with MatmulPerfMode.DoubleRow (2 k-planes of 128
    contracted per pass = 2x bf16 PE throughput); int codes 0..15 are exact
    in e4m3, x and the code*scale products round at ~3% which lands ~4e-4
    on the output. The precision-critical low-rank path (xr = x @ R^T, then
    xr @ L^T) stays bf16, keeping total rel err ~3e-3 like the bf16 kernel.
  * All dequantization happens on-device (DVE): codes arrive as exact
    fp8/bf16 values, scales arrive pre-broadcast along partitions; a DVE
    multiply produces dequantized weights (in-place for Q/R/L).
  * Per core: prologue computes xr^T = R @ x^T on the PE (bf16, out-features
    of xr on PSUM partitions so no transpose is ever needed) and casts x to
    fp8; the main loop walks 4 out-feature quarters x 8 token groups, each
    PSUM group = 2 bf16 low-rank matmuls + 16x4 fp8 DoubleRow base matmuls,
    bias fused into the DVE eviction.
  * Q codes+scales (16.8 MB each, fp8) stream through a 2-quarter SBUF ring
    overlapped with compute; x streams per k-tile in the prologue.

PE budget/core: 65K cyc (xr^T) + 65K (low-rank) + 524K (base fp8) = 655K
cyc = 273 us at 2.4 GHz, vs 1.05M cyc (437 us) for the all-bf16 kernel.
"""

import numpy as np
import ml_dtypes

P = 128
D_IN = 4096
D_OUT = 4096
TOK = 8192
RANK = 256
NCORES = 8
TPC = TOK // NCORES       # 1024 tokens per core
KT = D_IN // P            # 32 k-tiles
K2 = KT // 2              # 16 double-k-tiles (DoubleRow contracts 256)
NQ = 4                    # out-feature quarters
QW = D_OUT // NQ          # 1024
OCC = 512                 # psum chunk width (one bank)
TG = TPC // P             # 8 token groups
RT = RANK // P            # 2 rank tiles

_module_cache = {}
last_result = None


def _build_module():
    import concourse.mybir as mybir
    import concourse.tile as tile
    from concourse import bacc

    f32 = mybir.dt.float32
    bf16 = mybir.dt.bfloat16
    fp8 = mybir.dt.float8e4
    DR = mybir.MatmulPerfMode.DoubleRow

    nc = bacc.Bacc(None, target_bir_lowering=False, debug=False)
    # chunked layouts are partition-major within each chunk so one DMA
    # fills an SBUF tile slice with matching AP iteration order
    xb_d = nc.dram_tensor("xb", (KT // 4, P, 4, TPC), bf16,
                          kind="ExternalInput")
    qv_d = nc.dram_tensor("qv", (NQ, K2 // 4, P, 4, 2, QW), fp8,
                          kind="ExternalInput")
    qs_d = nc.dram_tensor("qs", (NQ, K2 // 4, P, 4, 2, QW), fp8,
                          kind="ExternalInput")
    rv_d = nc.dram_tensor("rv", (P, KT, RANK), bf16, kind="ExternalInput")
    rs_d = nc.dram_tensor("rs", (P, KT, RANK), bf16, kind="ExternalInput")
    lv_d = nc.dram_tensor("lv", (P, RT, D_OUT), bf16, kind="ExternalInput")
    ls_d = nc.dram_tensor("ls", (P, RT, D_OUT), bf16, kind="ExternalInput")
    bias_d = nc.dram_tensor("biasv", (P, D_OUT), f32, kind="ExternalInput")
    y_d = nc.dram_tensor("y", (TPC, D_OUT), f32, kind="ExternalOutput")

    with tile.TileContext(nc) as tc:
        with (
            tc.tile_pool(name="const", bufs=1) as const,
            tc.tile_pool(name="xbp", bufs=2) as xbp,
            tc.tile_pool(name="qp", bufs=2) as qp,
            tc.tile_pool(name="qsp", bufs=2) as qsp,
            tc.tile_pool(name="yp", bufs=4) as yp,
            tc.tile_pool(name="xrps", bufs=4, space="PSUM") as xrps,
            tc.tile_pool(name="pp", bufs=4, space="PSUM") as pp,
        ):
            rv = const.tile([P, KT, RANK], bf16)    # becomes dequantized R^T
            lv = const.tile([P, RT, D_OUT], bf16)   # becomes dequantized L^T
            bias_t = const.tile([P, D_OUT], f32)
            xrT = const.tile([P, RT, TPC], bf16)    # xr^T: [rank, tokens]
            x8 = const.tile([P, K2, 2, TPC], fp8)
            # prologue scratch staged in the Q-quarter ring (slots are
            # reclaimed by quarters 0/1 once the scales are consumed)
            rs = qp.tile([P, KT, RANK], bf16, tag="q", name="rs")
            ls = qp.tile([P, RT, D_OUT], bf16, tag="q", name="ls")

            # R goes on the fast HW queue (gates the first xr matmuls);
            # L/bias are not needed until ~35us in, so they ride the slow
            # gpsimd SW queue, keeping the HW queues clear for x and Q.
            nc.sync.dma_start(rv[:], rv_d[:])
            nc.scalar.dma_start(rs[:], rs_d[:])
            nc.gpsimd.dma_start(lv[:], lv_d[:])
            nc.gpsimd.dma_start(ls[:], ls_d[:])
            nc.gpsimd.dma_start(bias_t[:], bias_d[:])

            nc.vector.tensor_mul(rv[:], rv[:], rs[:])
            nc.vector.tensor_mul(lv[:], lv[:], ls[:])

            # quarter tiles allocated up-front in ring order (rs, ls, q0..q3);
            # DMAs/dequants are emitted interleaved with compute below.
            qts = [
                qp.tile([P, K2, 2, QW], fp8, tag="q", name=f"q{q}")
                for q in range(NQ)
            ]
            qsts = {}

            def fetch_chunk(q, c):
                # one 1MB DMA per 4 k2-tiles of codes and of scales
                nc.sync.dma_start(qts[q][:, 4 * c:4 * c + 4], qv_d[q, c])
                qst = qsp.tile([P, 4, 2, QW], fp8, tag="qs", name=f"qs{q}_{c}")
                nc.scalar.dma_start(qst[:], qs_d[q, c])
                qsts[(q, c)] = qst

            def dequant(q, k2):
                nc.vector.tensor_mul(
                    qts[q][:, k2], qts[q][:, k2], qsts[(q, k2 // 4)][:, k2 % 4]
                )

            # ---- prologue: xr^T = R @ x^T (bf16) + cast x -> fp8 (on Act)
            xr_ps = [
                xrps.tile([P, OCC], f32, tag="xr", name=f"xr{i}")
                for i in range(4)
            ]
            for c in range(KT // 4):
                xbt = xbp.tile([P, 4, TPC], bf16, tag="xb")
                dma_x = nc.sync if c % 2 == 0 else nc.scalar
                dma_x.dma_start(xbt[:], xb_d[c])
                for kk in range(4):
                    k = 4 * c + kk
                    for ts in range(2):
                        for rt in range(RT):
                            nc.tensor.matmul(
                                xr_ps[rt * 2 + ts][:],
                                rv[:, k, rt * P:(rt + 1) * P],
                                xbt[:, kk, ts * OCC:(ts + 1) * OCC],
                                start=(k == 0), stop=(k == KT - 1),
                            )
                # chunk c covers k2 = 2c, 2c+1: same (k2, pl, tok) order
                nc.scalar.copy(x8[:, 2 * c:2 * c + 2], xbt[:])
            # quarter-0 stream + xr^T eviction (eviction first on the DVE:
            # the PE needs xrT only ~7us into the main loop, while the
            # dequant stream must stay ahead of the base matmuls)
            fetch_chunk(0, 0)
            fetch_chunk(0, 1)
            for rt in range(RT):
                for ts in range(2):
                    nc.vector.tensor_copy(
                        xrT[:, rt, ts * OCC:(ts + 1) * OCC],
                        xr_ps[rt * 2 + ts][:],
                    )
            fetch_chunk(0, 2)
            fetch_chunk(0, 3)
            for k2 in range(K2):
                dequant(0, k2)

            # ---- main: 4 quarters x 8 token groups; base matmuls first in
            # each psum group (start), low-rank last (stop); quarter q+1 is
            # fetched/dequanted two k2-tiles per token group so the in-order
            # DVE stays ahead of the PE.
            for q in range(NQ):
                for t in range(TG):
                    psums = [
                        pp.tile([P, OCC], f32, tag="ps", name=f"ps{q}_{t}_{i}")
                        for i in range(2)
                    ]
                    for k2 in range(K2):
                        for oc2 in range(2):
                            nc.tensor.matmul(
                                psums[oc2][:],
                                x8[:, k2, :, t * P:(t + 1) * P],
                                qts[q][:, k2, :, oc2 * OCC:(oc2 + 1) * OCC],
                                start=(k2 == 0), stop=False,
                                perf_mode=DR,
                            )
                    for oc2 in range(2):
                        for rt in range(RT):
                            nc.tensor.matmul(
                                psums[oc2][:],
                                xrT[:, rt, t * P:(t + 1) * P],
                                lv[:, rt, q * QW + oc2 * OCC:
                                   q * QW + (oc2 + 1) * OCC],
                                start=False, stop=(rt == RT - 1),
                            )
                    if q + 1 < NQ:
                        if t % 2 == 0:
                            fetch_chunk(q + 1, t // 2)
                        dequant(q + 1, 2 * t)
                        dequant(q + 1, 2 * t + 1)
                    yt = yp.tile([P, QW], f32, tag="y")
                    for oc2 in range(2):
                        nc.vector.tensor_add(
                            yt[:, oc2 * OCC:(oc2 + 1) * OCC], psums[oc2][:],
                            bias_t[:, q * QW + oc2 * OCC:
                                   q * QW + (oc2 + 1) * OCC],
                        )
                    nc.gpsimd.dma_start(
                        y_d[t * P:(t + 1) * P, q * QW:(q + 1) * QW], yt[:]
                    )

    nc.compile()
    return nc


def kernel(x, q_values, q_scales, l_values, l_scales, r_values, r_scales, bias,
           _trace=False):
    from concourse.bass_utils import run_bass_kernel_spmd

    if "mod" not in _module_cache:
        _module_cache["mod"] = _build_module()
    nc = _module_cache["mod"]

    bf = ml_dtypes.bfloat16
    f8 = ml_dtypes.float8_e4m3
    x = np.asarray(x, np.float32)
    qv = np.asarray(q_values)
    qsc = np.asarray(q_scales, np.float32)
    lvv = np.asarray(l_values)
    lsc = np.asarray(l_scales, np.float32)
    rvv = np.asarray(r_values)
    rsc = np.asarray(r_scales, np.float32)
    b = np.asarray(bias, np.float32)

    # host-side marshaling (layout + dtype only; all dequant/matmul math
    # runs on-device). d_in index i = k2*256 + pl*128 + p throughout.
    # [q, c, p, kk, pl, oq] with k2 = 4c + kk, d_in i = k2*256 + pl*128 + p
    qv_h = np.ascontiguousarray(
        qv.T.reshape(K2 // 4, 4, 2, P, NQ, QW).transpose(4, 0, 3, 1, 2, 5)
    ).astype(np.float32).astype(f8)
    qs_small = (
        qsc.T.reshape(K2 // 4, 4, 2, NQ, QW).transpose(3, 0, 1, 2, 4).astype(f8)
    )
    qs_h = np.ascontiguousarray(
        np.broadcast_to(qs_small[:, :, None, :, :, :], (NQ, K2 // 4, P, 4, 2, QW))
    )
    rv_h = np.ascontiguousarray(
        rvv.T.reshape(KT, P, RANK).transpose(1, 0, 2)
    ).astype(np.float32).astype(bf)
    rs_h = np.ascontiguousarray(
        np.broadcast_to(rsc.T.astype(bf)[None, :, :], (P, KT, RANK))
    )
    lv_h = np.ascontiguousarray(
        lvv.T.reshape(RT, P, D_OUT).transpose(1, 0, 2)
    ).astype(np.float32).astype(bf)
    ls_h = np.ascontiguousarray(
        np.broadcast_to(lsc.T.astype(bf)[None, :, :], (P, RT, D_OUT))
    )
    bias_h = np.ascontiguousarray(np.broadcast_to(b[None, :], (P, D_OUT)))

    in_maps = []
    for c in range(NCORES):
        xb_h = np.ascontiguousarray(
            x[c * TPC:(c + 1) * TPC].T.reshape(KT // 4, 4, P, TPC)
            .transpose(0, 2, 1, 3)
        ).astype(bf)
        in_maps.append({
            "xb": xb_h, "qv": qv_h, "qs": qs_h, "rv": rv_h, "rs": rs_h,
            "lv": lv_h, "ls": ls_h, "biasv": bias_h,
        })

    res = run_bass_kernel_spmd(
        nc, in_maps, core_ids=list(range(NCORES)), trace=_trace
    )
    global last_result
    last_result = res
    return np.concatenate([r["y"] for r in res.results], axis=0)
